# revision 24
# baseline (speedup 1.0000x reference)
"""GCN (DGL GraphConv norm='both', 5 stacked layers) on 8 Trainium2 NeuronCores.

Strategy (1D node partition; v2 part/pass pipeline):
  - Nodes sharded contiguously across 8 cores (6250 nodes/core); edges
    partitioned by dst core. The small [5,128,128] weights are replicated.
  - The per-core node shard is cut into P=4 source "parts" (by row range);
    the scaled features hs = h * deg_out^-1/2 (fp16) are AllGathered per part
    into per-core full replicas, with each part's collective emitted as soon
    as its rows are produced, so the 4 AGs of the next layer overlap the
    current layer's compute.
  - Dst tiles are processed in 2 passes (tiles 0..24 / 25..48). Per layer the
    8 gather streams (pass x part) run on the gpsimd engine in part-arrival
    order so the in-order Pool queue never blocks on a late AllGather.
  - segment_sum over dst is a PE matmul against host-precomputed one-hot fp8
    selection blocks. Chunks are 128-edge windows of the (tile, src)-sorted
    stream; a chunk may span adjacent dst tiles (one matmul+sel block per
    (chunk, tile) touch), which removes per-tile chunk padding. Per-tile
    partial sums accumulate in PSUM within a stream and are added into a
    persistent SBUF accumulator (DVE) across the 4 parts.
  - Dense part: h = relu(deg_in^-1/2 * (agg @ W) + b) with the bias folded
    into the PSUM accumulation as an outer product inv_nd (x) b, and both
    norms folded into the relu's per-partition scale.

Host-side preprocessing only touches the static graph structure (degrees,
edge ordering, index/selection tables); all per-layer tensor math runs on
device.
"""

import hashlib

import numpy as np

import concourse.bass as bass
import concourse.mybir as mybir
import concourse.tile as tile
from concourse import bacc
from concourse.bass_utils import run_bass_kernel_spmd

N = 50000
E = 800000
D = 128
L = 5
NCORES = 8
NPC = N // NCORES          # 6250 nodes per core
TP = 128                   # tile partition (dst nodes per tile)
NT = (NPC + TP - 1) // TP  # 49 dst tiles per core (last has 106 rows)
LAST_ROWS = NPC - TP * (NT - 1)

# Source parts (row ranges of each core's shard) and dst passes (tile ranges).
PT = [0, 13, 25, 37, 49]                      # part boundaries in tiles
NPARTS = len(PT) - 1
PR = [min(PT[i + 1] * TP, NPC) - PT[i] * TP for i in range(NPARTS)]
PSTART = [PT[i] * TP for i in range(NPARTS)]
PASSES = [(0, 25), (25, NT)]

F32 = mybir.dt.float32
F16 = mybir.dt.float16
F8 = mybir.dt.float8e4
I16 = mybir.dt.int16

RG = [list(range(NCORES))]

LAST_EXEC_NS = None

# debug-only: subsystems to skip when building the program (timing attribution)
DEBUG_SKIP = set()

# tunables for the gather path
GATHER_SINGLE_PACKET = True
NQ = 4          # SWDGE queues to spread gathers across (ucode max 4)
GCAP = 1024     # max idxs per dma_gather call (SWDGE ring capacity)

_CACHE = {}


def _cdiv(a, b):
    return -(-a // b)


def _part_of_rows(loc):
    return (
        (loc >= PSTART[1]).astype(np.int64)
        + (loc >= PSTART[2]).astype(np.int64)
        + (loc >= PSTART[3]).astype(np.int64)
    )


def _make_schedule(src, dst):
    """Bake the (core-shared) stream/chunk/matmul schedule from the edges."""
    core = dst // NPC
    loc = dst % NPC
    t = loc // TP
    dl = loc % TP
    s_pass = (t >= PASSES[1][0]).astype(np.int64)
    srcloc = src % NPC
    srccore = src // NPC
    p_part = _part_of_rows(srcloc)

    # stream id per edge: (core, pass, part)
    sid = (core * 2 + s_pass) * NPARTS + p_part
    order = np.lexsort((srcloc, t, sid))
    o_sid = sid[order]
    o_t = t[order]
    o_dl = dl[order]
    o_srcloc = srcloc[order]
    o_srccore = srccore[order]
    o_part = p_part[order]

    nstream = NCORES * 2 * NPARTS
    cnt = np.bincount(o_sid, minlength=nstream).reshape(NCORES, 2 * NPARTS)
    # K per (pass, part): max over cores, rounded to chunks of 128
    Ksp = (_cdiv(cnt.max(axis=0), 128) * 128).reshape(2, NPARTS)
    CH = Ksp // 128

    # position of each edge within its stream
    starts = np.zeros(nstream + 1, np.int64)
    np.cumsum(cnt.reshape(-1), out=starts[1:])
    pos = np.arange(E) - starts[o_sid]
    chunk = pos // 128
    slot = pos % 128

    # per (pass, part, chunk): union of tiles over cores -> matmul ops
    ops = {}          # (s, p) -> list of (chunk, tile, start, stop)
    nblk = {}         # (s, p) -> number of sel blocks
    blk_of = {}       # (s, p, chunk, tile) -> block index within stream
    for s in range(2):
        for p in range(NPARTS):
            ntile_chunks = set()
            m = (o_part == p) & ((o_t >= PASSES[s][0]) & (o_t < PASSES[s][1]))
            cc_ = chunk[m]
            tt_ = o_t[m]
            pairs = np.unique(cc_ * 64 + tt_)
            chs = pairs // 64
            tls = pairs % 64
            # order ops by (chunk, tile)
            opl = sorted(zip(chs.tolist(), tls.tolist()))
            first = {}
            last = {}
            for i, (c, tt) in enumerate(opl):
                if tt not in first:
                    first[tt] = i
                last[tt] = i
            lst = []
            for i, (c, tt) in enumerate(opl):
                lst.append((int(c), int(tt), first[tt] == i, last[tt] == i))
                blk_of[(s, p, int(c), int(tt))] = i
            ops[(s, p)] = lst
            nblk[(s, p)] = len(lst)

    # idx/sel table offsets per stream (shared across cores)
    idxcol = np.zeros((2, NPARTS), np.int64)
    selbase = np.zeros((2, NPARTS), np.int64)
    ic = 0
    sb = 0
    for s in range(2):
        for p in range(NPARTS):
            idxcol[s, p] = ic
            selbase[s, p] = sb
            ic += int(Ksp[s, p]) // 16
            sb += nblk[(s, p)]
    return dict(
        Ksp=Ksp, CH=CH, ops=ops, nblk=nblk, blk_of=blk_of,
        idxcol=idxcol, selbase=selbase, ICOLS=ic, TOTBLK=sb,
        order=order, o_sid=o_sid, o_t=o_t, o_dl=o_dl, o_srcloc=o_srcloc,
        o_srccore=o_srccore, o_part=o_part, chunk=chunk, slot=slot,
        s_pass_sorted=(o_t >= PASSES[1][0]).astype(np.int64),
    )


def _make_core_inputs(sched, feat, src, dst, W, b):
    import ml_dtypes

    Ksp, idxcol, selbase, blk_of = (
        sched["Ksp"], sched["idxcol"], sched["selbase"], sched["blk_of"])
    ICOLS, TOTBLK = sched["ICOLS"], sched["TOTBLK"]

    deg_out = np.maximum(np.bincount(src, minlength=N), 1.0)
    deg_in = np.maximum(np.bincount(dst, minlength=N), 1.0)
    ns = (deg_out ** -0.5).astype(np.float32)
    nd = (deg_in ** -0.5).astype(np.float32)
    inv_nd = (1.0 / nd).astype(np.float32)

    o_sid = sched["o_sid"]
    o_t = sched["o_t"]
    o_dl = sched["o_dl"]
    o_srcloc = sched["o_srcloc"]
    o_srccore = sched["o_srccore"]
    o_part = sched["o_part"]
    chunk = sched["chunk"]
    slot = sched["slot"]
    s_sorted = sched["s_pass_sorted"]

    scc = o_sid // (2 * NPARTS)

    # idx value: row within the part's AllGather buffer
    prr = np.array([PR[p] for p in range(NPARTS)], np.int64)
    pst = np.array(PSTART, np.int64)
    idxval = (o_srccore * prr[o_part] + (o_srcloc - pst[o_part])).astype(np.int16)

    # idx table column/row for each edge
    icol_e = idxcol[s_sorted, o_part]
    i_in_stream = chunk * 128 + slot
    col = icol_e + i_in_stream // 16
    row = i_in_stream % 16

    # sel block per edge (vectorized lookup via unique keys)
    key = ((s_sorted * NPARTS + o_part) * 4096 + chunk) * 64 + o_t
    uk, inv = np.unique(key, return_inverse=True)
    uk_s = uk // (4096 * 64) // NPARTS
    uk_p = uk // (4096 * 64) % NPARTS
    uk_c = uk // 64 % 4096
    uk_t = uk % 64
    uk_blk = np.array(
        [blk_of[(int(a), int(b), int(c), int(d))]
         for a, b, c, d in zip(uk_s, uk_p, uk_c, uk_t)], np.int64)
    uk_selbase = selbase[uk_s, uk_p]
    blk = (uk_selbase + uk_blk)[inv]
    selcol = blk * 128 + o_dl

    w_all = np.ascontiguousarray(
        np.concatenate([W[l] for l in range(L)], axis=1), dtype=np.float32
    )
    b_all = np.ascontiguousarray(b[:L].reshape(1, L * D), dtype=np.float32)

    # ns per prologue tile: column j = rows of the j-th (part, core, sub) tile
    # in the part-major layer-0 layout (ceil(PR[p]/TP) sub-tiles per (p, core))
    nsub = [_cdiv(PR[p], TP) for p in range(NPARTS)]
    ncol0 = NCORES * sum(nsub)
    ns0f = np.zeros((TP, ncol0), np.float32)
    j = 0
    for p in range(NPARTS):
        for c in range(NCORES):
            base = c * NPC + PSTART[p]
            for su in range(nsub[p]):
                r0 = base + su * TP
                r = min(TP, base + PR[p] - r0)
                ns0f[0:r, j] = ns[r0:r0 + r]
                j += 1

    pad = NT * TP - NPC
    per_core = []
    for c in range(NCORES):
        m = scc == c
        idx_arr = np.zeros((16, ICOLS), np.int16)
        idx_arr[row[m], col[m]] = idxval[m]
        idx_arr = np.tile(idx_arr, (8, 1))
        sel_arr = np.zeros((128, TOTBLK * 128), ml_dtypes.float8_e4m3)
        sel_arr[slot[m], selcol[m]] = 1.0

        lo = c * NPC
        ndp = np.pad(nd[lo:lo + NPC], (0, pad)).reshape(NT, TP).T.copy()
        ndns = np.pad((nd * ns)[lo:lo + NPC], (0, pad)).reshape(NT, TP).T.copy()
        invndp = np.pad(inv_nd[lo:lo + NPC], (0, pad)).reshape(1, NT * TP).copy()

        per_core.append({
            "feat_f": np.ascontiguousarray(feat, dtype=np.float32),
            "idx": idx_arr,
            "sel": sel_arr,
            "w": w_all,
            "bb": b_all,
            "sc_mid": np.ascontiguousarray(ndns, dtype=np.float32),
            "sc_last": np.ascontiguousarray(ndp, dtype=np.float32),
            "invnd": invndp,
            "ns0f": np.ascontiguousarray(ns0f, dtype=np.float32),
        })
    return per_core


def _build_program(sched):
    Ksp, CH, ops, nblk = sched["Ksp"], sched["CH"], sched["ops"], sched["nblk"]
    idxcol, selbase = sched["idxcol"], sched["selbase"]
    ICOLS, TOTBLK = sched["ICOLS"], sched["TOTBLK"]

    # every tile must get a p==0 copy into aggSB (else stale data would leak)
    for s in range(2):
        tiles0 = {t for (_, t, _, _) in ops[(s, 0)]}
        assert tiles0 >= set(range(*PASSES[s])), (s, tiles0)
    # PSUM pool depth = max concurrently-live per-tile accumulators
    maxlive = 1
    for lst in ops.values():
        live = 0
        for (_, _, is_start, is_stop) in lst:
            if is_start:
                live += 1
                maxlive = max(maxlive, live)
            if is_stop:
                live -= 1
    psA_bufs = maxlive + 2
    assert psA_bufs <= 16, psA_bufs

    nsub = [_cdiv(PR[p], TP) for p in range(NPARTS)]
    ncol0 = NCORES * sum(nsub)

    nc = bacc.Bacc("TRN2", target_bir_lowering=False, debug=False,
                   num_devices=NCORES, num_swdge_queues=NQ)
    feat_in = nc.declare_dram_parameter("feat_f", [N, D], F32, isOutput=False)
    idx_in = nc.declare_dram_parameter("idx", [128, ICOLS], I16, isOutput=False)
    sel_in = nc.declare_dram_parameter("sel", [128, TOTBLK * 128], F8, isOutput=False)
    w_in = nc.declare_dram_parameter("w", [D, L * D], F32, isOutput=False)
    b_in = nc.declare_dram_parameter("bb", [1, L * D], F32, isOutput=False)
    scmid_in = nc.declare_dram_parameter("sc_mid", [TP, NT], F32, isOutput=False)
    sclast_in = nc.declare_dram_parameter("sc_last", [TP, NT], F32, isOutput=False)
    invnd_in = nc.declare_dram_parameter("invnd", [1, NT * TP], F32, isOutput=False)
    ns0_in = nc.declare_dram_parameter("ns0f", [TP, ncol0], F32, isOutput=False)
    out_ext = nc.declare_dram_parameter("out", [NPC, D], F32, isOutput=True)

    Relu = mybir.ActivationFunctionType.Relu

    with tile.TileContext(nc) as tc:
        with (
            tc.tile_pool(name="dramp", bufs=1, space="DRAM") as dp,
            tc.tile_pool(name="const", bufs=1) as cp,
            tc.tile_pool(name="aggp", bufs=1) as ap_,
            tc.tile_pool(name="gatp", bufs=2) as gpool,
            tc.tile_pool(name="selp", bufs=2) as spool,
            tc.tile_pool(name="workp", bufs=4) as wpool,
            tc.tile_pool(name="iop", bufs=3) as iop,
            tc.tile_pool(name="psA", bufs=psA_bufs, space="PSUM") as pA,
            tc.tile_pool(name="psB", bufs=3, space="PSUM") as pB,
        ):
            # layer 0 reads a locally-built part-laid-out feat*ns (no AG);
            # hsP[1..L-1] are the AllGather landing buffers
            hs0P = [
                dp.tile([NCORES * PR[p], D], F16, name=f"hs0P_{p}", bufs=1)
                for p in range(NPARTS)
            ]
            hsP = [None] + [
                [dp.tile([NCORES * PR[p], D], F16, addr_space="Shared",
                         name=f"hsP{i}_{p}", bufs=1) for p in range(NPARTS)]
                for i in range(1, L)
            ]
            bnP = [
                [dp.tile([PR[p], D], F16, name=f"bn{p}_{w}", bufs=1)
                 for w in (0, 1)]
                for p in range(NPARTS)
            ]

            idx_sb = cp.tile([128, ICOLS], I16)
            nc.sync.dma_start(out=idx_sb[:, :], in_=idx_in[:, :])
            w_sb = cp.tile([D, L * D], F32)
            nc.sync.dma_start(out=w_sb[:, :], in_=w_in[:, :])
            b_sb = cp.tile([1, L * D], F32)
            nc.sync.dma_start(out=b_sb[:, :], in_=b_in[:, :])
            scmid_sb = cp.tile([TP, NT], F32)
            nc.sync.dma_start(out=scmid_sb[:, :], in_=scmid_in[:, :])
            sclast_sb = cp.tile([TP, NT], F32)
            nc.sync.dma_start(out=sclast_sb[:, :], in_=sclast_in[:, :])
            invnd_sb = cp.tile([1, NT * TP], F32)
            nc.sync.dma_start(out=invnd_sb[:, :], in_=invnd_in[:, :])
            ns0_sb = cp.tile([TP, ncol0], F32)
            nc.sync.dma_start(out=ns0_sb[:, :], in_=ns0_in[:, :])

            aggSB = ap_.tile([128, NT * TP], F32)

            qctr = [0]
            kreg = {}
            for s in range(2):
                for p in range(NPARTS):
                    K = int(Ksp[s, p])
                    while K > 0:
                        piece = min(K, GCAP)
                        if piece not in kreg:
                            kreg[piece] = nc.gpsimd.to_reg(piece)
                        K -= piece

            def rows_of(t):
                return TP if t < NT - 1 else LAST_ROWS

            def part_of_tile(t):
                for p in range(NPARTS):
                    if t < PT[p + 1]:
                        return p
                raise AssertionError

            def bounce_out(t, r, src_ap, which):
                p = part_of_tile(t)
                b0 = (t - PT[p]) * TP
                nc.sync.dma_start(out=bnP[p][which][b0:b0 + r, :], in_=src_ap)

            def emit_cc(p, which, lnext):
                nc.gpsimd.collective_compute(
                    "AllGather", mybir.AluOpType.bypass, replica_groups=RG,
                    ins=[bnP[p][which].opt()], outs=[hsP[lnext][p].opt()],
                )

            # ---- prologue: hs0 = feat * ns built locally in part-major order
            # from the full feat input (no collective on the startup path)
            j0 = 0
            for p in range(NPARTS):
                for c in range(NCORES):
                    base = c * NPC + PSTART[p]
                    for su in range(nsub[p]):
                        r0 = base + su * TP
                        r = min(TP, base + PR[p] - r0)
                        ft = iop.tile([TP, D], F32, tag="ft")
                        nc.sync.dma_start(out=ft[0:r, :], in_=feat_in[r0:r0 + r, :])
                        h0 = iop.tile([TP, D], F16, tag="h0")
                        nc.vector.tensor_scalar_mul(
                            h0[0:r, :], ft[0:r, :], ns0_sb[0:r, j0:j0 + 1])
                        d0 = c * PR[p] + su * TP
                        nc.sync.dma_start(
                            out=hs0P[p][d0:d0 + r, :], in_=h0[0:r, :])
                        j0 += 1

            def phase_b(t, l):
                r = rows_of(t)
                ps2 = pB.tile([TP, D], F32, tag="ps2")
                nc.tensor.matmul(
                    ps2[:, :], aggSB[:, t * TP:(t + 1) * TP],
                    w_sb[:, l * D:(l + 1) * D],
                    start=True, stop=False,
                )
                nc.tensor.matmul(
                    ps2[:, :],
                    invnd_sb[0:1, t * TP:(t + 1) * TP],
                    b_sb[0:1, l * D:(l + 1) * D],
                    start=False, stop=True,
                )
                if l < L - 1:
                    hn = wpool.tile([TP, D], F16, tag="hsn")
                    nc.scalar.activation(
                        hn[0:r, :], ps2[0:r, :], Relu,
                        scale=scmid_sb[0:r, t:t + 1],
                    )
                    bounce_out(t, r, hn[0:r, :], (l + 1) % 2)
                    p = part_of_tile(t)
                    # under cc-skip keep layer 0's AGs so hsP[1] has a writer
                    if t == PT[p + 1] - 1 and (l == 0 or "cc" not in DEBUG_SKIP):
                        emit_cc(p, (l + 1) % 2, l + 1)
                else:
                    hf = wpool.tile([TP, D], F32, tag="hfin")
                    nc.scalar.activation(
                        hf[0:r, :], ps2[0:r, :], Relu,
                        scale=sclast_sb[0:r, t:t + 1],
                    )
                    nc.sync.dma_start(
                        out=out_ext[t * TP:t * TP + r, :], in_=hf[0:r, :]
                    )

            # ---- layers
            for l in range(L):
                li = max(1, l) if "cc" in DEBUG_SKIP else l
                for s in range(2):
                    for p in range(NPARTS):
                        K = int(Ksp[s, p])
                        CHsp = K // 128
                        gt = gpool.tile([128, CHsp, D], F16, tag="gat")
                        if "gather" in DEBUG_SKIP:
                            nc.vector.memset(gt[:, 0:1, :], 0.0)
                        icol = int(idxcol[s, p])
                        hsrc = hs0P[p] if l == 0 else hsP[li][p]
                        done = 0
                        while done < K and "gather" not in DEBUG_SKIP:
                            piece = min(K - done, GCAP)
                            c0, c1 = done // 128, (done + piece) // 128
                            nc.gpsimd.dma_gather(
                                gt[:, c0:c1, :],
                                hsrc[:, :],
                                idx_sb[:, icol + done // 16:icol + (done + piece) // 16],
                                piece, kreg[piece], D,
                                queue_num=qctr[0] % NQ,
                                single_packet=GATHER_SINGLE_PACKET,
                            )
                            qctr[0] += 1
                            done += piece
                        nb = nblk[(s, p)]
                        st = spool.tile([128, nb * 128], F8, tag="sel")
                        sb0 = int(selbase[s, p])
                        nc.scalar.dma_start(
                            out=st[:, :],
                            in_=sel_in[:, sb0 * 128:(sb0 + nb) * 128],
                        )
                        psums = {}
                        for bi, (c, t, is_start, is_stop) in enumerate(ops[(s, p)]):
                            if is_start:
                                psT = pA.tile([D, TP], F32, tag="psT")
                                psums[t] = psT
                            nc.tensor.matmul(
                                psums[t][:, :],
                                gt[:, c, :],
                                st[:, bi * 128:(bi + 1) * 128],
                                start=is_start, stop=is_stop,
                            )
                            if is_stop:
                                sl = aggSB[:, t * TP:(t + 1) * TP]
                                if p == 0:
                                    nc.vector.tensor_copy(out=sl, in_=psums[t][:, :])
                                else:
                                    nc.vector.tensor_add(sl, psums[t][:, :], sl)
                                del psums[t]
                    # dense phase for this pass (emits next layer's part AGs)
                    for t in range(*PASSES[s]):
                        phase_b(t, l)
    nc.compile()
    return nc


def _get_compiled(src, dst):
    dig = hashlib.sha256(src.tobytes() + dst.tobytes()).hexdigest()
    if dig not in _CACHE:
        sched = _make_schedule(src, dst)
        nc = _build_program(sched)
        _CACHE[dig] = (sched, nc)
    return _CACHE[dig]


def kernel(feat, src, dst, W, b, trace=False):
    global LAST_EXEC_NS
    feat = np.asarray(feat, dtype=np.float32)
    src = np.asarray(src).astype(np.int64)
    dst = np.asarray(dst).astype(np.int64)
    W = np.asarray(W, dtype=np.float32)
    b = np.asarray(b, dtype=np.float32)

    sched, nc = _get_compiled(src, dst)
    in_maps = _make_core_inputs(sched, feat, src, dst, W, b)
    res = run_bass_kernel_spmd(nc, in_maps, list(range(NCORES)), trace=trace)
    LAST_EXEC_NS = res.exec_time_ns
    out = np.concatenate([res.results[c]["out"] for c in range(NCORES)], axis=0)
    return out.astype(np.float32)


# revision 25
# speedup vs baseline: 1.2012x; 1.2012x over previous
"""GCN (DGL GraphConv norm='both', 5 stacked layers) on 8 Trainium2 NeuronCores.

Strategy (1D node partition; v2 part/pass pipeline):
  - Nodes sharded contiguously across 8 cores (6250 nodes/core); edges
    partitioned by dst core. The small [5,128,128] weights are replicated.
  - The per-core node shard is cut into P=4 source "parts" (by row range);
    the scaled features hs = h * deg_out^-1/2 (fp16) are AllGathered per part
    into per-core full replicas, with each part's collective emitted as soon
    as its rows are produced, so the 4 AGs of the next layer overlap the
    current layer's compute.
  - Dst tiles are processed in 2 passes (tiles 0..24 / 25..48). Per layer the
    8 gather streams (pass x part) run on the gpsimd engine in part-arrival
    order so the in-order Pool queue never blocks on a late AllGather.
  - segment_sum over dst is a PE matmul against host-precomputed one-hot fp8
    selection blocks. Chunks are 128-edge windows of the (tile, src)-sorted
    stream; a chunk may span adjacent dst tiles (one matmul+sel block per
    (chunk, tile) touch), which removes per-tile chunk padding. Per-tile
    partial sums accumulate in PSUM within a stream and are added into a
    persistent SBUF accumulator (DVE) across the 4 parts.
  - Dense part: h = relu(deg_in^-1/2 * (agg @ W) + b) with the bias folded
    into the PSUM accumulation as an outer product inv_nd (x) b, and both
    norms folded into the relu's per-partition scale.

Host-side preprocessing only touches the static graph structure (degrees,
edge ordering, index/selection tables); all per-layer tensor math runs on
device.
"""

import hashlib

import numpy as np

import concourse.bass as bass
import concourse.mybir as mybir
import concourse.tile as tile
from concourse import bacc
from concourse.bass_utils import run_bass_kernel_spmd

N = 50000
E = 800000
D = 128
L = 5
NCORES = 8
NPC = N // NCORES          # 6250 nodes per core
TP = 128                   # tile partition (dst nodes per tile)
NT = (NPC + TP - 1) // TP  # 49 dst tiles per core (last has 106 rows)
LAST_ROWS = NPC - TP * (NT - 1)

# Source parts (row ranges of each core's shard) and dst passes (tile ranges).
PT = [0, 13, 25, 37, 49]                      # part boundaries in tiles
NPARTS = len(PT) - 1
PR = [min(PT[i + 1] * TP, NPC) - PT[i] * TP for i in range(NPARTS)]
PSTART = [PT[i] * TP for i in range(NPARTS)]
PASSES = [(0, 25), (25, NT)]

F32 = mybir.dt.float32
F16 = mybir.dt.float16
F8 = mybir.dt.float8e4
I16 = mybir.dt.int16

RG = [list(range(NCORES))]

LAST_EXEC_NS = None

# debug-only: subsystems to skip when building the program (timing attribution)
DEBUG_SKIP = set()

# tunables for the gather path
GATHER_SINGLE_PACKET = True
NQ = 4          # SWDGE queues to spread gathers across (ucode max 4)
GCAP = 1024     # max idxs per dma_gather call (SWDGE ring capacity)

_CACHE = {}


def _cdiv(a, b):
    return -(-a // b)


def _part_of_rows(loc):
    return (
        (loc >= PSTART[1]).astype(np.int64)
        + (loc >= PSTART[2]).astype(np.int64)
        + (loc >= PSTART[3]).astype(np.int64)
    )


def _make_schedule(src, dst):
    """Bake the (core-shared) stream/chunk/matmul schedule from the edges."""
    core = dst // NPC
    loc = dst % NPC
    t = loc // TP
    dl = loc % TP
    s_pass = (t >= PASSES[1][0]).astype(np.int64)
    srcloc = src % NPC
    srccore = src // NPC
    p_part = _part_of_rows(srcloc)

    # stream id per edge: (core, pass, part)
    sid = (core * 2 + s_pass) * NPARTS + p_part
    order = np.lexsort((srcloc, t, sid))
    o_sid = sid[order]
    o_t = t[order]
    o_dl = dl[order]
    o_srcloc = srcloc[order]
    o_srccore = srccore[order]
    o_part = p_part[order]

    nstream = NCORES * 2 * NPARTS
    cnt = np.bincount(o_sid, minlength=nstream).reshape(NCORES, 2 * NPARTS)
    # K per (pass, part): max over cores, rounded to chunks of 128
    Ksp = (_cdiv(cnt.max(axis=0), 128) * 128).reshape(2, NPARTS)
    CH = Ksp // 128

    # position of each edge within its stream
    starts = np.zeros(nstream + 1, np.int64)
    np.cumsum(cnt.reshape(-1), out=starts[1:])
    pos = np.arange(E) - starts[o_sid]
    chunk = pos // 128
    slot = pos % 128

    # per (pass, part, chunk): union of tiles over cores -> matmul ops
    ops = {}          # (s, p) -> list of (chunk, tile, start, stop)
    nblk = {}         # (s, p) -> number of sel blocks
    blk_of = {}       # (s, p, chunk, tile) -> block index within stream
    for s in range(2):
        for p in range(NPARTS):
            ntile_chunks = set()
            m = (o_part == p) & ((o_t >= PASSES[s][0]) & (o_t < PASSES[s][1]))
            cc_ = chunk[m]
            tt_ = o_t[m]
            pairs = np.unique(cc_ * 64 + tt_)
            chs = pairs // 64
            tls = pairs % 64
            # order ops by (chunk, tile)
            opl = sorted(zip(chs.tolist(), tls.tolist()))
            first = {}
            last = {}
            for i, (c, tt) in enumerate(opl):
                if tt not in first:
                    first[tt] = i
                last[tt] = i
            lst = []
            for i, (c, tt) in enumerate(opl):
                lst.append((int(c), int(tt), first[tt] == i, last[tt] == i))
                blk_of[(s, p, int(c), int(tt))] = i
            ops[(s, p)] = lst
            nblk[(s, p)] = len(lst)

    # idx/sel table offsets per stream (shared across cores)
    idxcol = np.zeros((2, NPARTS), np.int64)
    selbase = np.zeros((2, NPARTS), np.int64)
    ic = 0
    sb = 0
    for s in range(2):
        for p in range(NPARTS):
            idxcol[s, p] = ic
            selbase[s, p] = sb
            ic += int(Ksp[s, p]) // 16
            sb += nblk[(s, p)]
    return dict(
        Ksp=Ksp, CH=CH, ops=ops, nblk=nblk, blk_of=blk_of,
        idxcol=idxcol, selbase=selbase, ICOLS=ic, TOTBLK=sb,
        order=order, o_sid=o_sid, o_t=o_t, o_dl=o_dl, o_srcloc=o_srcloc,
        o_srccore=o_srccore, o_part=o_part, chunk=chunk, slot=slot,
        s_pass_sorted=(o_t >= PASSES[1][0]).astype(np.int64),
    )


def _make_core_inputs(sched, feat, src, dst, W, b):
    import ml_dtypes

    Ksp, idxcol, selbase, blk_of = (
        sched["Ksp"], sched["idxcol"], sched["selbase"], sched["blk_of"])
    ICOLS, TOTBLK = sched["ICOLS"], sched["TOTBLK"]

    deg_out = np.maximum(np.bincount(src, minlength=N), 1.0)
    deg_in = np.maximum(np.bincount(dst, minlength=N), 1.0)
    ns = (deg_out ** -0.5).astype(np.float32)
    nd = (deg_in ** -0.5).astype(np.float32)
    inv_nd = (1.0 / nd).astype(np.float32)

    o_sid = sched["o_sid"]
    o_t = sched["o_t"]
    o_dl = sched["o_dl"]
    o_srcloc = sched["o_srcloc"]
    o_srccore = sched["o_srccore"]
    o_part = sched["o_part"]
    chunk = sched["chunk"]
    slot = sched["slot"]
    s_sorted = sched["s_pass_sorted"]

    scc = o_sid // (2 * NPARTS)

    # idx value: row within the part's AllGather buffer
    prr = np.array([PR[p] for p in range(NPARTS)], np.int64)
    pst = np.array(PSTART, np.int64)
    idxval = (o_srccore * prr[o_part] + (o_srcloc - pst[o_part])).astype(np.int16)

    # idx table column/row for each edge
    icol_e = idxcol[s_sorted, o_part]
    i_in_stream = chunk * 128 + slot
    col = icol_e + i_in_stream // 16
    row = i_in_stream % 16

    # sel block per edge (vectorized lookup via unique keys)
    key = ((s_sorted * NPARTS + o_part) * 4096 + chunk) * 64 + o_t
    uk, inv = np.unique(key, return_inverse=True)
    uk_s = uk // (4096 * 64) // NPARTS
    uk_p = uk // (4096 * 64) % NPARTS
    uk_c = uk // 64 % 4096
    uk_t = uk % 64
    uk_blk = np.array(
        [blk_of[(int(a), int(b), int(c), int(d))]
         for a, b, c, d in zip(uk_s, uk_p, uk_c, uk_t)], np.int64)
    uk_selbase = selbase[uk_s, uk_p]
    blk = (uk_selbase + uk_blk)[inv]
    selcol = blk * 128 + o_dl

    w_all = np.ascontiguousarray(
        np.concatenate([W[l] for l in range(L)], axis=1), dtype=np.float32
    )
    b_all = np.ascontiguousarray(b[:L].reshape(1, L * D), dtype=np.float32)

    pad = NT * TP - NPC
    per_core = []
    for c in range(NCORES):
        m = scc == c
        idx_arr = np.zeros((16, ICOLS), np.int16)
        idx_arr[row[m], col[m]] = idxval[m]
        idx_arr = np.tile(idx_arr, (8, 1))
        sel_arr = np.zeros((128, TOTBLK * 128), ml_dtypes.float8_e4m3)
        sel_arr[slot[m], selcol[m]] = 1.0

        lo = c * NPC
        ndp = np.pad(nd[lo:lo + NPC], (0, pad)).reshape(NT, TP).T.copy()
        ndns = np.pad((nd * ns)[lo:lo + NPC], (0, pad)).reshape(NT, TP).T.copy()
        invndp = np.pad(inv_nd[lo:lo + NPC], (0, pad)).reshape(1, NT * TP).copy()
        nsp = np.pad(ns[lo:lo + NPC], (0, pad)).reshape(NT, TP).T.copy()

        per_core.append({
            "feat_s": np.ascontiguousarray(feat[lo:lo + NPC], dtype=np.float32),
            "idx": idx_arr,
            "sel": sel_arr,
            "w": w_all,
            "bb": b_all,
            "sc_mid": np.ascontiguousarray(ndns, dtype=np.float32),
            "sc_last": np.ascontiguousarray(ndp, dtype=np.float32),
            "invnd": invndp,
            "ns0": np.ascontiguousarray(nsp, dtype=np.float32),
        })
    return per_core


def _build_program(sched):
    Ksp, CH, ops, nblk = sched["Ksp"], sched["CH"], sched["ops"], sched["nblk"]
    idxcol, selbase = sched["idxcol"], sched["selbase"]
    ICOLS, TOTBLK = sched["ICOLS"], sched["TOTBLK"]

    # every tile must get a p==0 copy into aggSB (else stale data would leak)
    for s in range(2):
        tiles0 = {t for (_, t, _, _) in ops[(s, 0)]}
        assert tiles0 >= set(range(*PASSES[s])), (s, tiles0)
    # PSUM pool depth = max concurrently-live per-tile accumulators
    maxlive = 1
    for lst in ops.values():
        live = 0
        for (_, _, is_start, is_stop) in lst:
            if is_start:
                live += 1
                maxlive = max(maxlive, live)
            if is_stop:
                live -= 1
    psA_bufs = maxlive + 2
    assert psA_bufs <= 16, psA_bufs

    nc = bacc.Bacc("TRN2", target_bir_lowering=False, debug=False,
                   num_devices=NCORES, num_swdge_queues=NQ)
    feat_in = nc.declare_dram_parameter("feat_s", [NPC, D], F32, isOutput=False)
    idx_in = nc.declare_dram_parameter("idx", [128, ICOLS], I16, isOutput=False)
    sel_in = nc.declare_dram_parameter("sel", [128, TOTBLK * 128], F8, isOutput=False)
    w_in = nc.declare_dram_parameter("w", [D, L * D], F32, isOutput=False)
    b_in = nc.declare_dram_parameter("bb", [1, L * D], F32, isOutput=False)
    scmid_in = nc.declare_dram_parameter("sc_mid", [TP, NT], F32, isOutput=False)
    sclast_in = nc.declare_dram_parameter("sc_last", [TP, NT], F32, isOutput=False)
    invnd_in = nc.declare_dram_parameter("invnd", [1, NT * TP], F32, isOutput=False)
    ns0_in = nc.declare_dram_parameter("ns0", [TP, NT], F32, isOutput=False)
    out_ext = nc.declare_dram_parameter("out", [NPC, D], F32, isOutput=True)

    Relu = mybir.ActivationFunctionType.Relu

    with tile.TileContext(nc) as tc:
        with (
            tc.tile_pool(name="dramp", bufs=1, space="DRAM") as dp,
            tc.tile_pool(name="const", bufs=1) as cp,
            tc.tile_pool(name="aggp", bufs=1) as ap_,
            tc.tile_pool(name="gatp", bufs=2) as gpool,
            tc.tile_pool(name="selp", bufs=2) as spool,
            tc.tile_pool(name="workp", bufs=4) as wpool,
            tc.tile_pool(name="iop", bufs=3) as iop,
            tc.tile_pool(name="psA", bufs=psA_bufs, space="PSUM") as pA,
            tc.tile_pool(name="psB", bufs=3, space="PSUM") as pB,
        ):
            hsP = [
                [dp.tile([NCORES * PR[p], D], F16, addr_space="Shared",
                         name=f"hsP{i}_{p}", bufs=1) for p in range(NPARTS)]
                for i in range(L)
            ]
            bnP = [
                [dp.tile([PR[p], D], F16, name=f"bn{p}_{w}", bufs=1)
                 for w in (0, 1)]
                for p in range(NPARTS)
            ]

            idx_sb = cp.tile([128, ICOLS], I16)
            nc.sync.dma_start(out=idx_sb[:, :], in_=idx_in[:, :])
            w_sb = cp.tile([D, L * D], F32)
            nc.sync.dma_start(out=w_sb[:, :], in_=w_in[:, :])
            b_sb = cp.tile([1, L * D], F32)
            nc.sync.dma_start(out=b_sb[:, :], in_=b_in[:, :])
            scmid_sb = cp.tile([TP, NT], F32)
            nc.sync.dma_start(out=scmid_sb[:, :], in_=scmid_in[:, :])
            sclast_sb = cp.tile([TP, NT], F32)
            nc.sync.dma_start(out=sclast_sb[:, :], in_=sclast_in[:, :])
            invnd_sb = cp.tile([1, NT * TP], F32)
            nc.sync.dma_start(out=invnd_sb[:, :], in_=invnd_in[:, :])
            ns0_sb = cp.tile([TP, NT], F32)
            nc.sync.dma_start(out=ns0_sb[:, :], in_=ns0_in[:, :])

            aggSB = ap_.tile([128, NT * TP], F32)

            qctr = [0]
            kreg = {}
            for s in range(2):
                for p in range(NPARTS):
                    K = int(Ksp[s, p])
                    while K > 0:
                        piece = min(K, GCAP)
                        if piece not in kreg:
                            kreg[piece] = nc.gpsimd.to_reg(piece)
                        K -= piece

            def rows_of(t):
                return TP if t < NT - 1 else LAST_ROWS

            def part_of_tile(t):
                for p in range(NPARTS):
                    if t < PT[p + 1]:
                        return p
                raise AssertionError

            def bounce_out(t, r, src_ap, which):
                p = part_of_tile(t)
                b0 = (t - PT[p]) * TP
                nc.sync.dma_start(out=bnP[p][which][b0:b0 + r, :], in_=src_ap)

            def emit_cc(p, which, lnext):
                nc.gpsimd.collective_compute(
                    "AllGather", mybir.AluOpType.bypass, replica_groups=RG,
                    ins=[bnP[p][which].opt()], outs=[hsP[lnext][p].opt()],
                )

            # ---- prologue: hs0 = feat * ns, shard -> bounce0 -> AllGather/part
            for t in range(NT):
                r = rows_of(t)
                ft = iop.tile([TP, D], F32, tag="ft")
                nc.sync.dma_start(out=ft[0:r, :], in_=feat_in[t * TP:t * TP + r, :])
                h0 = iop.tile([TP, D], F16, tag="h0")
                nc.vector.tensor_scalar_mul(h0[0:r, :], ft[0:r, :], ns0_sb[0:r, t:t + 1])
                bounce_out(t, r, h0[0:r, :], 0)
                p = part_of_tile(t)
                if t == PT[p + 1] - 1:
                    emit_cc(p, 0, 0)

            def phase_b(t, l):
                r = rows_of(t)
                ps2 = pB.tile([TP, D], F32, tag="ps2")
                nc.tensor.matmul(
                    ps2[:, :], aggSB[:, t * TP:(t + 1) * TP],
                    w_sb[:, l * D:(l + 1) * D],
                    start=True, stop=False,
                )
                nc.tensor.matmul(
                    ps2[:, :],
                    invnd_sb[0:1, t * TP:(t + 1) * TP],
                    b_sb[0:1, l * D:(l + 1) * D],
                    start=False, stop=True,
                )
                if l < L - 1:
                    hn = wpool.tile([TP, D], F16, tag="hsn")
                    nc.scalar.activation(
                        hn[0:r, :], ps2[0:r, :], Relu,
                        scale=scmid_sb[0:r, t:t + 1],
                    )
                    bounce_out(t, r, hn[0:r, :], (l + 1) % 2)
                    p = part_of_tile(t)
                    if t == PT[p + 1] - 1 and "cc" not in DEBUG_SKIP:
                        emit_cc(p, (l + 1) % 2, l + 1)
                else:
                    hf = wpool.tile([TP, D], F32, tag="hfin")
                    nc.scalar.activation(
                        hf[0:r, :], ps2[0:r, :], Relu,
                        scale=sclast_sb[0:r, t:t + 1],
                    )
                    nc.sync.dma_start(
                        out=out_ext[t * TP:t * TP + r, :], in_=hf[0:r, :]
                    )

            # ---- layers
            for l in range(L):
                li = 0 if "cc" in DEBUG_SKIP else l
                for s in range(2):
                    for p in range(NPARTS):
                        K = int(Ksp[s, p])
                        CHsp = K // 128
                        gt = gpool.tile([128, CHsp, D], F16, tag="gat")
                        if "gather" in DEBUG_SKIP:
                            nc.vector.memset(gt[:, 0:1, :], 0.0)
                        icol = int(idxcol[s, p])
                        done = 0
                        while done < K and "gather" not in DEBUG_SKIP:
                            piece = min(K - done, GCAP)
                            c0, c1 = done // 128, (done + piece) // 128
                            nc.gpsimd.dma_gather(
                                gt[:, c0:c1, :],
                                hsP[li][p][:, :],
                                idx_sb[:, icol + done // 16:icol + (done + piece) // 16],
                                piece, kreg[piece], D,
                                queue_num=qctr[0] % NQ,
                                single_packet=GATHER_SINGLE_PACKET,
                            )
                            qctr[0] += 1
                            done += piece
                        nb = nblk[(s, p)]
                        st = spool.tile([128, nb * 128], F8, tag="sel")
                        sb0 = int(selbase[s, p])
                        nc.scalar.dma_start(
                            out=st[:, :],
                            in_=sel_in[:, sb0 * 128:(sb0 + nb) * 128],
                        )
                        psums = {}
                        for bi, (c, t, is_start, is_stop) in enumerate(ops[(s, p)]):
                            if is_start:
                                psT = pA.tile([D, TP], F32, tag="psT")
                                psums[t] = psT
                            nc.tensor.matmul(
                                psums[t][:, :],
                                gt[:, c, :],
                                st[:, bi * 128:(bi + 1) * 128],
                                start=is_start, stop=is_stop,
                            )
                            if is_stop:
                                sl = aggSB[:, t * TP:(t + 1) * TP]
                                if p == 0:
                                    nc.vector.tensor_copy(out=sl, in_=psums[t][:, :])
                                else:
                                    nc.vector.tensor_add(sl, psums[t][:, :], sl)
                                del psums[t]
                    # dense phase for this pass (emits next layer's part AGs)
                    for t in range(*PASSES[s]):
                        phase_b(t, l)
    nc.compile()
    return nc


def _get_compiled(src, dst):
    dig = hashlib.sha256(src.tobytes() + dst.tobytes()).hexdigest()
    if dig not in _CACHE:
        sched = _make_schedule(src, dst)
        nc = _build_program(sched)
        _CACHE[dig] = (sched, nc)
    return _CACHE[dig]


def kernel(feat, src, dst, W, b, trace=False):
    global LAST_EXEC_NS
    feat = np.asarray(feat, dtype=np.float32)
    src = np.asarray(src).astype(np.int64)
    dst = np.asarray(dst).astype(np.int64)
    W = np.asarray(W, dtype=np.float32)
    b = np.asarray(b, dtype=np.float32)

    sched, nc = _get_compiled(src, dst)
    in_maps = _make_core_inputs(sched, feat, src, dst, W, b)
    res = run_bass_kernel_spmd(nc, in_maps, list(range(NCORES)), trace=trace)
    LAST_EXEC_NS = res.exec_time_ns
    out = np.concatenate([res.results[c]["out"] for c in range(NCORES)], axis=0)
    return out.astype(np.float32)


# revision 30
# speedup vs baseline: 1.2822x; 1.0674x over previous
"""GCN (DGL GraphConv norm='both', 5 stacked layers) on 8 Trainium2 NeuronCores.

Strategy (1D node partition; v2 part/pass pipeline):
  - Nodes sharded contiguously across 8 cores (6250 nodes/core); edges
    partitioned by dst core. The small [5,128,128] weights are replicated.
  - The per-core node shard is cut into P=4 source "parts" (by row range);
    the scaled features hs = h * deg_out^-1/2 (fp16) are AllGathered per part
    into per-core full replicas, with each part's collective emitted as soon
    as its rows are produced, so the 4 AGs of the next layer overlap the
    current layer's compute.
  - Dst tiles are processed in 2 passes (tiles 0..24 / 25..48). Per layer the
    8 gather streams (pass x part) run on the gpsimd engine in part-arrival
    order so the in-order Pool queue never blocks on a late AllGather.
  - segment_sum over dst is a PE matmul against host-precomputed one-hot fp8
    selection blocks. Chunks are 128-edge windows of the (tile, src)-sorted
    stream; a chunk may span adjacent dst tiles (one matmul+sel block per
    (chunk, tile) touch), which removes per-tile chunk padding. Per-tile
    partial sums accumulate in PSUM within a stream and are added into a
    persistent SBUF accumulator (DVE) across the 4 parts.
  - Dense part: h = relu(deg_in^-1/2 * (agg @ W) + b) with the bias folded
    into the PSUM accumulation as an outer product inv_nd (x) b, and both
    norms folded into the relu's per-partition scale.

Host-side preprocessing only touches the static graph structure (degrees,
edge ordering, index/selection tables); all per-layer tensor math runs on
device.
"""

import hashlib

import numpy as np

import concourse.bass as bass
import concourse.mybir as mybir
import concourse.tile as tile
from concourse import bacc
from concourse.bass_utils import run_bass_kernel_spmd

N = 50000
E = 800000
D = 128
L = 5
NCORES = 8
NPC = N // NCORES          # 6250 nodes per core
TP = 128                   # tile partition (dst nodes per tile)
NT = (NPC + TP - 1) // TP  # 49 dst tiles per core (last has 106 rows)
LAST_ROWS = NPC - TP * (NT - 1)

# Source parts (row ranges of each core's shard) and dst passes (tile ranges).
PT = [0, 13, 25, 37, 49]                      # part boundaries in tiles
NPARTS = len(PT) - 1
PR = [min(PT[i + 1] * TP, NPC) - PT[i] * TP for i in range(NPARTS)]
PSTART = [PT[i] * TP for i in range(NPARTS)]
PASSES = [(0, 25), (25, NT)]

F32 = mybir.dt.float32
F16 = mybir.dt.float16
F8 = mybir.dt.float8e4
I16 = mybir.dt.int16

RG = [list(range(NCORES))]

LAST_EXEC_NS = None

# debug-only: subsystems to skip when building the program (timing attribution)
DEBUG_SKIP = set()

# tunables for the gather path
GATHER_SINGLE_PACKET = True
NQ = 4          # SWDGE queues to spread gathers across (ucode max 4)
GCAP = 1024     # max idxs per dma_gather call (SWDGE ring capacity)

_CACHE = {}


def _cdiv(a, b):
    return -(-a // b)


def _part_of_rows(loc):
    return (
        (loc >= PSTART[1]).astype(np.int64)
        + (loc >= PSTART[2]).astype(np.int64)
        + (loc >= PSTART[3]).astype(np.int64)
    )


def _make_schedule(src, dst):
    """Bake the (core-shared) stream/chunk/matmul schedule from the edges."""
    core = dst // NPC
    loc = dst % NPC
    t = loc // TP
    dl = loc % TP
    s_pass = (t >= PASSES[1][0]).astype(np.int64)
    srcloc = src % NPC
    srccore = src // NPC
    p_part = _part_of_rows(srcloc)

    # stream id per edge: (core, pass, part)
    sid = (core * 2 + s_pass) * NPARTS + p_part
    order = np.lexsort((srccore, srcloc, t, sid))
    o_sid = sid[order]
    o_t = t[order]
    o_dl = dl[order]
    o_srcloc = srcloc[order]
    o_srccore = srccore[order]
    o_part = p_part[order]

    # Dedup gather slots by (stream, tile, src): a src row with several edges
    # into the same dst tile is gathered once; its sel block covers all the
    # dst columns (with multiplicity as the value for exact dup edges).
    new = np.r_[True,
                (o_sid[1:] != o_sid[:-1])
                | (o_t[1:] != o_t[:-1])
                | (o_srcloc[1:] != o_srcloc[:-1])
                | (o_srccore[1:] != o_srccore[:-1])]
    uidx = np.cumsum(new) - 1          # edge -> unique slot
    u_sid = o_sid[new]
    u_t = o_t[new]
    u_srcloc = o_srcloc[new]
    u_srccore = o_srccore[new]
    u_part = o_part[new]
    NU = len(u_sid)

    nstream = NCORES * 2 * NPARTS
    cnt = np.bincount(u_sid, minlength=nstream).reshape(NCORES, 2 * NPARTS)
    # K per (pass, part): max over cores, rounded to chunks of 128
    Ksp = (_cdiv(cnt.max(axis=0), 128) * 128).reshape(2, NPARTS)
    CH = Ksp // 128

    # position of each unique slot within its stream
    starts = np.zeros(nstream + 1, np.int64)
    np.cumsum(cnt.reshape(-1), out=starts[1:])
    upos = np.arange(NU) - starts[u_sid]
    uchunk = upos // 128
    uslot = upos % 128
    chunk = uchunk[uidx]   # per-edge: chunk/slot of its unique
    slot = uslot[uidx]

    # per (pass, part, chunk): union of tiles over cores -> matmul ops
    ops = {}          # (s, p) -> list of (chunk, tile, start, stop)
    nblk = {}         # (s, p) -> number of sel blocks
    blk_of = {}       # (s, p, chunk, tile) -> block index within stream
    u_s = (u_t >= PASSES[1][0]).astype(np.int64)
    for s in range(2):
        for p in range(NPARTS):
            m = (u_part == p) & (u_s == s)
            cc_ = uchunk[m]
            tt_ = u_t[m]
            pairs = np.unique(cc_ * 64 + tt_)
            chs = pairs // 64
            tls = pairs % 64
            # order ops by (chunk, tile)
            opl = sorted(zip(chs.tolist(), tls.tolist()))
            first = {}
            last = {}
            for i, (c, tt) in enumerate(opl):
                if tt not in first:
                    first[tt] = i
                last[tt] = i
            lst = []
            for i, (c, tt) in enumerate(opl):
                lst.append((int(c), int(tt), first[tt] == i, last[tt] == i))
                blk_of[(s, p, int(c), int(tt))] = i
            ops[(s, p)] = lst
            nblk[(s, p)] = len(lst)

    # idx/sel table offsets per stream (shared across cores)
    idxcol = np.zeros((2, NPARTS), np.int64)
    selbase = np.zeros((2, NPARTS), np.int64)
    ic = 0
    sb = 0
    for s in range(2):
        for p in range(NPARTS):
            idxcol[s, p] = ic
            selbase[s, p] = sb
            ic += int(Ksp[s, p]) // 16
            sb += nblk[(s, p)]
    return dict(
        Ksp=Ksp, CH=CH, ops=ops, nblk=nblk, blk_of=blk_of,
        idxcol=idxcol, selbase=selbase, ICOLS=ic, TOTBLK=sb,
        order=order, o_sid=o_sid, o_t=o_t, o_dl=o_dl, o_srcloc=o_srcloc,
        o_srccore=o_srccore, o_part=o_part, chunk=chunk, slot=slot,
        s_pass_sorted=(o_t >= PASSES[1][0]).astype(np.int64),
        u_sid=u_sid, u_t=u_t, u_srcloc=u_srcloc, u_srccore=u_srccore,
        u_part=u_part, u_s=u_s, uchunk=uchunk, uslot=uslot,
    )


def _make_core_inputs(sched, feat, src, dst, W, b):
    import ml_dtypes

    Ksp, idxcol, selbase, blk_of = (
        sched["Ksp"], sched["idxcol"], sched["selbase"], sched["blk_of"])
    ICOLS, TOTBLK = sched["ICOLS"], sched["TOTBLK"]

    deg_out = np.maximum(np.bincount(src, minlength=N), 1.0)
    deg_in = np.maximum(np.bincount(dst, minlength=N), 1.0)
    ns = (deg_out ** -0.5).astype(np.float32)
    nd = (deg_in ** -0.5).astype(np.float32)
    inv_nd = (1.0 / nd).astype(np.float32)

    o_sid = sched["o_sid"]
    o_t = sched["o_t"]
    o_dl = sched["o_dl"]
    o_srcloc = sched["o_srcloc"]
    o_srccore = sched["o_srccore"]
    o_part = sched["o_part"]
    chunk = sched["chunk"]
    slot = sched["slot"]
    s_sorted = sched["s_pass_sorted"]

    scc = o_sid // (2 * NPARTS)
    u_sid = sched["u_sid"]
    u_part = sched["u_part"]
    u_s = sched["u_s"]
    uchunk = sched["uchunk"]
    uslot = sched["uslot"]
    scc_u = u_sid // (2 * NPARTS)

    # idx value per unique slot: row within the part's AllGather buffer
    prr = np.array([PR[p] for p in range(NPARTS)], np.int64)
    pst = np.array(PSTART, np.int64)
    idxval = (sched["u_srccore"] * prr[u_part]
              + (sched["u_srcloc"] - pst[u_part])).astype(np.int16)

    # idx table column/row for each unique slot
    icol_u = idxcol[u_s, u_part]
    i_in_stream = uchunk * 128 + uslot
    col = icol_u + i_in_stream // 16
    row = i_in_stream % 16

    # sel block per edge (vectorized lookup via unique keys)
    key = ((s_sorted * NPARTS + o_part) * 4096 + chunk) * 64 + o_t
    uk, inv = np.unique(key, return_inverse=True)
    uk_s = uk // (4096 * 64) // NPARTS
    uk_p = uk // (4096 * 64) % NPARTS
    uk_c = uk // 64 % 4096
    uk_t = uk % 64
    uk_blk = np.array(
        [blk_of[(int(a), int(b), int(c), int(d))]
         for a, b, c, d in zip(uk_s, uk_p, uk_c, uk_t)], np.int64)
    uk_selbase = selbase[uk_s, uk_p]
    blk = (uk_selbase + uk_blk)[inv]
    selcol = blk * 128 + o_dl

    w_all = np.ascontiguousarray(
        np.concatenate([W[l] for l in range(L)], axis=1), dtype=np.float32
    )
    b_all = np.ascontiguousarray(b[:L].reshape(1, L * D), dtype=np.float32)

    pad = NT * TP - NPC
    per_core = []
    for c in range(NCORES):
        m = scc == c
        mu = scc_u == c
        idx_arr = np.zeros((16, ICOLS), np.int16)
        idx_arr[row[mu], col[mu]] = idxval[mu]
        idx_arr = np.tile(idx_arr, (8, 1))
        self_f32 = np.zeros((128, TOTBLK * 128), np.float32)
        np.add.at(self_f32, (slot[m], selcol[m]), 1.0)
        sel_arr = self_f32.astype(ml_dtypes.float8_e4m3)

        lo = c * NPC
        ndp = np.pad(nd[lo:lo + NPC], (0, pad)).reshape(NT, TP).T.copy()
        ndns = np.pad((nd * ns)[lo:lo + NPC], (0, pad)).reshape(NT, TP).T.copy()
        invndp = np.pad(inv_nd[lo:lo + NPC], (0, pad)).reshape(1, NT * TP).copy()
        nsp = np.pad(ns[lo:lo + NPC], (0, pad)).reshape(NT, TP).T.copy()

        per_core.append({
            "feat_s": np.ascontiguousarray(feat[lo:lo + NPC], dtype=np.float32),
            "idx": idx_arr,
            "sel": sel_arr,
            "w": w_all,
            "bb": b_all,
            "sc_mid": np.ascontiguousarray(ndns, dtype=np.float32),
            "sc_last": np.ascontiguousarray(ndp, dtype=np.float32),
            "invnd": invndp,
            "ns0": np.ascontiguousarray(nsp, dtype=np.float32),
        })
    return per_core


def _build_program(sched):
    Ksp, CH, ops, nblk = sched["Ksp"], sched["CH"], sched["ops"], sched["nblk"]
    idxcol, selbase = sched["idxcol"], sched["selbase"]
    ICOLS, TOTBLK = sched["ICOLS"], sched["TOTBLK"]

    # every tile must get a p==0 copy into aggSB (else stale data would leak)
    for s in range(2):
        tiles0 = {t for (_, t, _, _) in ops[(s, 0)]}
        assert tiles0 >= set(range(*PASSES[s])), (s, tiles0)
    # PSUM pool depth = max concurrently-live per-tile accumulators
    maxlive = 1
    for lst in ops.values():
        live = 0
        for (_, _, is_start, is_stop) in lst:
            if is_start:
                live += 1
                maxlive = max(maxlive, live)
            if is_stop:
                live -= 1
    psA_bufs = maxlive + 2
    assert psA_bufs <= 16, psA_bufs

    nc = bacc.Bacc("TRN2", target_bir_lowering=False, debug=False,
                   num_devices=NCORES, num_swdge_queues=NQ)
    feat_in = nc.declare_dram_parameter("feat_s", [NPC, D], F32, isOutput=False)
    idx_in = nc.declare_dram_parameter("idx", [128, ICOLS], I16, isOutput=False)
    sel_in = nc.declare_dram_parameter("sel", [128, TOTBLK * 128], F8, isOutput=False)
    w_in = nc.declare_dram_parameter("w", [D, L * D], F32, isOutput=False)
    b_in = nc.declare_dram_parameter("bb", [1, L * D], F32, isOutput=False)
    scmid_in = nc.declare_dram_parameter("sc_mid", [TP, NT], F32, isOutput=False)
    sclast_in = nc.declare_dram_parameter("sc_last", [TP, NT], F32, isOutput=False)
    invnd_in = nc.declare_dram_parameter("invnd", [1, NT * TP], F32, isOutput=False)
    ns0_in = nc.declare_dram_parameter("ns0", [TP, NT], F32, isOutput=False)
    out_ext = nc.declare_dram_parameter("out", [NPC, D], F32, isOutput=True)

    Relu = mybir.ActivationFunctionType.Relu

    with tile.TileContext(nc) as tc:
        with (
            tc.tile_pool(name="dramp", bufs=1, space="DRAM") as dp,
            tc.tile_pool(name="const", bufs=1) as cp,
            tc.tile_pool(name="aggp", bufs=1) as ap_,
            tc.tile_pool(name="gatp", bufs=2) as gpool,
            tc.tile_pool(name="selp", bufs=2) as spool,
            tc.tile_pool(name="workp", bufs=4) as wpool,
            tc.tile_pool(name="iop", bufs=3) as iop,
            tc.tile_pool(name="psA", bufs=psA_bufs, space="PSUM") as pA,
            tc.tile_pool(name="psB", bufs=3, space="PSUM") as pB,
        ):
            hsP = [
                [dp.tile([NCORES * PR[p], D], F16, addr_space="Shared",
                         name=f"hsP{i}_{p}", bufs=1) for p in range(NPARTS)]
                for i in range(L)
            ]
            bnP = [
                [dp.tile([PR[p], D], F16, name=f"bn{p}_{w}", bufs=1)
                 for w in (0, 1)]
                for p in range(NPARTS)
            ]

            idx_sb = cp.tile([128, ICOLS], I16)
            nc.sync.dma_start(out=idx_sb[:, :], in_=idx_in[:, :])
            w_sb = cp.tile([D, L * D], F32)
            nc.sync.dma_start(out=w_sb[:, :], in_=w_in[:, :])
            b_sb = cp.tile([1, L * D], F32)
            nc.sync.dma_start(out=b_sb[:, :], in_=b_in[:, :])
            scmid_sb = cp.tile([TP, NT], F32)
            nc.sync.dma_start(out=scmid_sb[:, :], in_=scmid_in[:, :])
            sclast_sb = cp.tile([TP, NT], F32)
            nc.sync.dma_start(out=sclast_sb[:, :], in_=sclast_in[:, :])
            invnd_sb = cp.tile([1, NT * TP], F32)
            nc.sync.dma_start(out=invnd_sb[:, :], in_=invnd_in[:, :])
            ns0_sb = cp.tile([TP, NT], F32)
            nc.sync.dma_start(out=ns0_sb[:, :], in_=ns0_in[:, :])

            aggSB = ap_.tile([128, NT * TP], F32)

            qctr = [0]
            kreg = {}
            for s in range(2):
                for p in range(NPARTS):
                    K = int(Ksp[s, p])
                    while K > 0:
                        piece = min(K, GCAP)
                        if piece not in kreg:
                            kreg[piece] = nc.gpsimd.to_reg(piece)
                        K -= piece

            def rows_of(t):
                return TP if t < NT - 1 else LAST_ROWS

            def part_of_tile(t):
                for p in range(NPARTS):
                    if t < PT[p + 1]:
                        return p
                raise AssertionError

            def bounce_out(t, r, src_ap, which):
                p = part_of_tile(t)
                b0 = (t - PT[p]) * TP
                nc.sync.dma_start(out=bnP[p][which][b0:b0 + r, :], in_=src_ap)

            def emit_cc(p, which, lnext):
                nc.gpsimd.collective_compute(
                    "AllGather", mybir.AluOpType.bypass, replica_groups=RG,
                    ins=[bnP[p][which].opt()], outs=[hsP[lnext][p].opt()],
                )

            # ---- prologue: hs0 = feat * ns, shard -> bounce0 -> AllGather/part
            for t in range(NT):
                r = rows_of(t)
                ft = iop.tile([TP, D], F32, tag="ft")
                nc.sync.dma_start(out=ft[0:r, :], in_=feat_in[t * TP:t * TP + r, :])
                h0 = iop.tile([TP, D], F16, tag="h0")
                nc.vector.tensor_scalar_mul(h0[0:r, :], ft[0:r, :], ns0_sb[0:r, t:t + 1])
                bounce_out(t, r, h0[0:r, :], 0)
                p = part_of_tile(t)
                if t == PT[p + 1] - 1:
                    emit_cc(p, 0, 0)

            def phase_b(t, l):
                r = rows_of(t)
                ps2 = pB.tile([TP, D], F32, tag="ps2")
                nc.tensor.matmul(
                    ps2[:, :], aggSB[:, t * TP:(t + 1) * TP],
                    w_sb[:, l * D:(l + 1) * D],
                    start=True, stop=False,
                )
                nc.tensor.matmul(
                    ps2[:, :],
                    invnd_sb[0:1, t * TP:(t + 1) * TP],
                    b_sb[0:1, l * D:(l + 1) * D],
                    start=False, stop=True,
                )
                if l < L - 1:
                    hn = wpool.tile([TP, D], F16, tag="hsn")
                    nc.scalar.activation(
                        hn[0:r, :], ps2[0:r, :], Relu,
                        scale=scmid_sb[0:r, t:t + 1],
                    )
                    bounce_out(t, r, hn[0:r, :], (l + 1) % 2)
                    p = part_of_tile(t)
                    if t == PT[p + 1] - 1 and "cc" not in DEBUG_SKIP:
                        emit_cc(p, (l + 1) % 2, l + 1)
                else:
                    hf = wpool.tile([TP, D], F32, tag="hfin")
                    nc.scalar.activation(
                        hf[0:r, :], ps2[0:r, :], Relu,
                        scale=sclast_sb[0:r, t:t + 1],
                    )
                    nc.sync.dma_start(
                        out=out_ext[t * TP:t * TP + r, :], in_=hf[0:r, :]
                    )

            # ---- layers
            for l in range(L):
                li = 0 if "cc" in DEBUG_SKIP else l
                for s in range(2):
                    for p in range(NPARTS):
                        K = int(Ksp[s, p])
                        CHsp = K // 128
                        gt = gpool.tile([128, CHsp, D], F16, tag="gat")
                        if "gather" in DEBUG_SKIP:
                            nc.vector.memset(gt[:, 0:1, :], 0.0)
                        icol = int(idxcol[s, p])
                        done = 0
                        while done < K and "gather" not in DEBUG_SKIP:
                            piece = min(K - done, GCAP)
                            c0, c1 = done // 128, (done + piece) // 128
                            nc.gpsimd.dma_gather(
                                gt[:, c0:c1, :],
                                hsP[li][p][:, :],
                                idx_sb[:, icol + done // 16:icol + (done + piece) // 16],
                                piece, kreg[piece], D,
                                queue_num=qctr[0] % NQ,
                                single_packet=GATHER_SINGLE_PACKET,
                            )
                            qctr[0] += 1
                            done += piece
                        nb = nblk[(s, p)]
                        st = spool.tile([128, nb * 128], F8, tag="sel")
                        sb0 = int(selbase[s, p])
                        nc.scalar.dma_start(
                            out=st[:, :],
                            in_=sel_in[:, sb0 * 128:(sb0 + nb) * 128],
                        )
                        psums = {}
                        for bi, (c, t, is_start, is_stop) in enumerate(ops[(s, p)]):
                            if is_start:
                                psT = pA.tile([D, TP], F32, tag="psT")
                                psums[t] = psT
                            nc.tensor.matmul(
                                psums[t][:, :],
                                gt[:, c, :],
                                st[:, bi * 128:(bi + 1) * 128],
                                start=is_start, stop=is_stop,
                            )
                            if is_stop:
                                sl = aggSB[:, t * TP:(t + 1) * TP]
                                if p == 0:
                                    nc.vector.tensor_copy(out=sl, in_=psums[t][:, :])
                                else:
                                    nc.vector.tensor_add(sl, psums[t][:, :], sl)
                                del psums[t]
                    # dense phase for this pass (emits next layer's part AGs)
                    for t in range(*PASSES[s]):
                        phase_b(t, l)
    nc.compile()
    return nc


def _get_compiled(src, dst):
    dig = hashlib.sha256(src.tobytes() + dst.tobytes()).hexdigest()
    if dig not in _CACHE:
        sched = _make_schedule(src, dst)
        nc = _build_program(sched)
        _CACHE[dig] = (sched, nc)
    return _CACHE[dig]


def kernel(feat, src, dst, W, b, trace=False):
    global LAST_EXEC_NS
    feat = np.asarray(feat, dtype=np.float32)
    src = np.asarray(src).astype(np.int64)
    dst = np.asarray(dst).astype(np.int64)
    W = np.asarray(W, dtype=np.float32)
    b = np.asarray(b, dtype=np.float32)

    sched, nc = _get_compiled(src, dst)
    in_maps = _make_core_inputs(sched, feat, src, dst, W, b)
    res = run_bass_kernel_spmd(nc, in_maps, list(range(NCORES)), trace=trace)
    LAST_EXEC_NS = res.exec_time_ns
    out = np.concatenate([res.results[c]["out"] for c in range(NCORES)], axis=0)
    return out.astype(np.float32)
